# revision 32
# baseline (speedup 1.0000x reference)
"""Trainium2 Bass kernel for LocalSparseAttention.

Problem (hardcoded): B=2, S=2048, D=1024, H=16, HD=64, WINDOW=128 (band
|i-j| <= 64), fp32 I/O.

Sharding: 8 cores = 2 batches x 4 head-groups (4 heads each). Each core:
  - qk projection into transposed layout [512, 2048] (head-pair packed)
  - v projection into natural layout, 19 (possibly 64-shifted) seq chunks
  - banded attention: per 128-query tile, 256-key window, additive mask
    applied via identity-matmul into PSUM, exp on ACT, AV + softmax
    denominator via ones-augmented v, normalization via PE broadcast
  - output projection -> fp16 partial [2048, 1024]
Host: fp16 casts/transposes in, sum of 4 partials per batch + fused bias
(b_out + b_v @ w_out) out.

All matmuls run in fp16 (1 cycle/row on PE, ~3e-4 rel err) with fp32 PSUM
accumulation; softmax exp input stays fp32.
"""
import sys

if "/opt/trn_rl_repo" not in sys.path:
    sys.path.insert(0, "/opt/trn_rl_repo")

import numpy as np

import concourse.bass as bass
import concourse.mybir as mybir
import concourse.tile as tile
from concourse import bacc
from concourse.bass_utils import run_bass_kernel_spmd

B, S, D, H, HD = 2, 2048, 1024, 16, 64
SCALE = HD**-0.5
C_SUB = 4.0  # subtracted from all scores via the mask; cancels in softmax
MASK_NEG = -30000.0

F16 = mybir.dt.float16
F32 = mybir.dt.float32
F32R = mybir.dt.float32r

# 19 key/value chunk offsets: 15 shifted (128c+64) + aligned 0,128,1792,1920
OFFS = [128 * c + 64 for c in range(15)] + [0, 128, 1792, 1920]


def _chunk_pair(i):
    if i == 0:
        return 15, 16
    if i == 15:
        return 17, 18
    return i - 1, i


def _mask_variant(i):
    return 0 if i == 0 else (2 if i == 15 else 1)


def _build_masks():
    kp = np.arange(128)[:, None]
    p = np.arange(128)[None, :]
    masks = np.zeros((128, 3, 2, 128), np.float16)
    for v, shift in enumerate([0, 64, 128]):
        for half in (0, 1):
            w = 128 * half + kp
            valid = np.abs(p + shift - w) <= 64
            masks[:, v, half, :] = np.where(valid, -C_SUB, MASK_NEG).astype(
                np.float16
            )
    return masks


def _build_program(stage=5, nc4=4, nhp=2):
    nc = bacc.Bacc("TRN2", debug=False, num_devices=8)

    xT_d = nc.dram_tensor("xT", [D, S], F16, kind="ExternalInput").ap()
    wqk_d = nc.dram_tensor("wqk", [D, 512], F16, kind="ExternalInput").ap()
    wv_d = nc.dram_tensor("wv", [D, 256], F16, kind="ExternalInput").ap()
    wout_d = nc.dram_tensor("wout", [256, D], F16, kind="ExternalInput").ap()
    bqk_d = nc.dram_tensor("bqk", [128, 4], F32, kind="ExternalInput").ap()
    masks_d = nc.dram_tensor("masks", [128, 3, 2, 128], F32,
                             kind="ExternalInput").ap()
    out_d = nc.dram_tensor("out", [S, D], F16, kind="ExternalOutput").ap()

    with tile.TileContext(nc) as tc:
        with (
            tc.tile_pool(name="const", bufs=1) as cpool,
            tc.tile_pool(name="work", bufs=2) as wpool,
            tc.tile_pool(name="expp", bufs=10) as epool,
            tc.tile_pool(name="scsp", bufs=6) as scpool,
            tc.tile_pool(name="ysb", bufs=3) as ypool,
            tc.tile_pool(name="ps512", bufs=2, space="PSUM") as ps512,
            tc.tile_pool(name="psv", bufs=2, space="PSUM") as psv,
            tc.tile_pool(name="pssc", bufs=2, space="PSUM") as pssc,
            tc.tile_pool(name="psav", bufs=2, space="PSUM") as psav,
        ):
            # ---- persistent SBUF tensors ----
            xT_sb = cpool.tile([128, 8, S], F16, tag="xT")
            wqk_sb = cpool.tile([128, 8, 512], F16, tag="wqk")
            wv_sb = cpool.tile([128, 8, 256], F16, tag="wv")
            wout_sb = cpool.tile([128, 2, D], F16, tag="wout")
            bqk_sb = cpool.tile([128, 4], F32, tag="bqk")
            masks_sb = cpool.tile([128, 3, 2, 128], F32, tag="masks")
            qk_sb = cpool.tile([128, 4, S], F16, tag="qk")
            v_sb = cpool.tile([128, 19, 4, 65], F16, tag="v")
            aoT_sb = cpool.tile([128, 2, S], F16, tag="aoT")
            ones_sb = cpool.tile([128, 64], F16, tag="ones")
            onescol_sb = cpool.tile([128, 1], F16, tag="onescol")

            # ---- input DMAs, split per k-chunk so compute starts early ----
            xT_r = xT_d.rearrange("(ko kp) s -> kp ko s", kp=128)
            wqk_r = wqk_d.rearrange("(ko kp) n -> kp ko n", kp=128)
            wv_r = wv_d.rearrange("(ko kp) n -> kp ko n", kp=128)
            for kt in range(8):
                nc.sync.dma_start(out=wqk_sb[:, kt], in_=wqk_r[:, kt])
                nc.sync.dma_start(out=wv_sb[:, kt], in_=wv_r[:, kt])
                nc.sync.dma_start(out=xT_sb[:, kt], in_=xT_r[:, kt])
            nc.sync.dma_start(out=bqk_sb[:], in_=bqk_d)
            nc.sync.dma_start(out=masks_sb[:], in_=masks_d)
            nc.sync.dma_start(
                out=wout_sb[:],
                in_=wout_d.rearrange("(t p) n -> p t n", p=128),
            )
            nc.vector.memset(ones_sb[:], 1.0)
            nc.vector.memset(onescol_sb[:], 1.0)
            nc.vector.memset(v_sb[:, :, :, 64:65], 1.0)

            # ---- stage B1: q/k projection  qkT[m*128+p, s] ----
            # m-tiles: 0,1 = q heads (0,1),(2,3) scaled; 2,3 = k heads
            for m in range(4):
                scale = SCALE if m < 2 else 1.0
                for ns in range(4):
                    ps = ps512.tile([128, 512], F32, tag="ps512")
                    for kt in range(8):
                        nc.tensor.matmul(
                            out=ps[:],
                            lhsT=wqk_sb[:, kt, m * 128:(m + 1) * 128],
                            rhs=xT_sb[:, kt, ns * 512:(ns + 1) * 512],
                            start=(kt == 0),
                            stop=(kt == 7),
                        )
                    nc.scalar.activation(
                        out=qk_sb[:, m, ns * 512:(ns + 1) * 512],
                        in_=ps[:],
                        func=mybir.ActivationFunctionType.Identity,
                        bias=bqk_sb[:, m:m + 1],
                        scale=scale,
                    )

            # ---- stage B2: v projection into 19 chunks, natural layout ----
            for c, off in enumerate(OFFS):
                ps = psv.tile([128, 256], F32, tag="psv")
                for kt in range(8):
                    nc.tensor.matmul(
                        out=ps[:],
                        lhsT=xT_sb[:, kt, off:off + 128],
                        rhs=wv_sb[:, kt, :],
                        start=(kt == 0),
                        stop=(kt == 7),
                    )
                nc.scalar.copy(
                    out=v_sb[:, c, :, 0:64],
                    in_=ps[:].rearrange("p (h d) -> p h d", h=4),
                )

            # ---- stage C: banded attention ----
            for c4 in range(nc4 if stage >= 2 else 0):
                for hp in range(nhp):
                    # separate PSUM tile (own accumulation group) per head:
                    # mixing lhsT row-groups 0/64 in one group breaks HW
                    exp_tiles = {}
                    for ii in range(4):
                        i = c4 * 4 + ii
                        cA, cB = _chunk_pair(i)
                        mv = _mask_variant(i)
                        for hh in range(2):
                            po = hh * 64
                            sc = pssc.tile([128, 2, 128], F32, tag="pssc")
                            for half, cc in enumerate((cA, cB)):
                                off = OFFS[cc]
                                nc.tensor.matmul(
                                    out=sc[:, half, :],
                                    lhsT=qk_sb[po:po + 64, 2 + hp,
                                               off:off + 128],
                                    rhs=qk_sb[po:po + 64, hp,
                                              i * 128:(i + 1) * 128],
                                    start=(half == 0),
                                    stop=(half == 1),
                                )
                            # mask add on DVE (drains PSUM early), exp on ACT
                            scs = scpool.tile([128, 2, 128], F32, tag="scs")
                            nc.vector.tensor_add(
                                out=scs[:], in0=sc[:], in1=masks_sb[:, mv]
                            )
                            ex = epool.tile([128, 2, 128], F16, tag="exp")
                            nc.scalar.activation(
                                out=ex[:],
                                in_=scs[:],
                                func=mybir.ActivationFunctionType.Exp,
                            )
                            exp_tiles[(ii, hh)] = ex

                    if stage < 3:
                        continue
                    av = {}
                    for hh in range(2):
                        h = 2 * hp + hh
                        avt = psav.tile([128, 4, 128], F32, tag="psav")
                        av[hh] = avt
                        for ii in range(4):
                            i = c4 * 4 + ii
                            cA, cB = _chunk_pair(i)
                            ex = exp_tiles[(ii, hh)]
                            first = ii == 0
                            last = ii == 3
                            if hh == 0:
                                # even head: ones-augmented v, denom in row 64
                                for half, cc in enumerate((cA, cB)):
                                    nc.tensor.matmul(
                                        out=avt[0:65, ii, :],
                                        lhsT=v_sb[:, cc, h, 0:65],
                                        rhs=ex[:, half, :],
                                        start=(first and half == 0),
                                        stop=(last and half == 1),
                                    )
                            else:
                                # odd head: data at partitions 64:128,
                                # denominator at partition 0 (its own
                                # partition-disjoint accumulation group)
                                for half, cc in enumerate((cA, cB)):
                                    nc.tensor.matmul(
                                        out=avt[64:128, ii, :],
                                        lhsT=v_sb[:, cc, h, 0:64],
                                        rhs=ex[:, half, :],
                                        start=(first and half == 0),
                                        stop=(last and half == 1),
                                    )
                                for half in range(2):
                                    nc.tensor.matmul(
                                        out=avt[0:1, ii, :],
                                        lhsT=onescol_sb[:],
                                        rhs=ex[:, half, :],
                                        start=(first and half == 0),
                                        stop=(last and half == 1),
                                    )

                    if stage < 4:
                        continue
                    # normalization: copy raw denoms to SBUF (f16), PE
                    # broadcast, approx-reciprocal on the broadcast, multiply
                    den = wpool.tile([65, 512], F16, tag="den")
                    nc.scalar.copy(
                        out=den[64:65, :],
                        in_=av[0][64:65, :, :].rearrange("p a b -> p (a b)"),
                    )
                    nc.scalar.copy(
                        out=den[0:1, :],
                        in_=av[1][0:1, :, :].rearrange("p a b -> p (a b)"),
                    )
                    bc = pssc.tile([128, 512], F32, tag="pssc")
                    bc2 = bc[:]
                    nc.tensor.matmul(
                        out=bc2[0:64, :], lhsT=ones_sb[64:65, :],
                        rhs=den[64:65, :], start=True, stop=True,
                    )
                    nc.tensor.matmul(
                        out=bc2[64:128, :], lhsT=ones_sb[0:1, :],
                        rhs=den[0:1, :], start=True, stop=True,
                    )
                    bcs = wpool.tile([128, 512], F32, tag="bcs")
                    nc.vector.reciprocal_approx_fast(out=bcs[:], in_=bc2)
                    sl = slice(c4 * 512, (c4 + 1) * 512)
                    nc.vector.tensor_mul(
                        out=aoT_sb[0:64, hp, sl],
                        in0=av[0][0:64, :, :].rearrange("p a b -> p (a b)"),
                        in1=bcs[0:64, :],
                    )
                    nc.vector.tensor_mul(
                        out=aoT_sb[64:128, hp, sl],
                        in0=av[1][64:128, :, :].rearrange("p a b -> p (a b)"),
                        in1=bcs[64:128, :],
                    )

                # ---- stage D (interleaved): out projection for this c4's
                # s-tiles — gives the PE dense independent work between
                # attention chains (keeps HAM warm)
                for st in range(c4 * 4, c4 * 4 + 4) if stage >= 5 else []:
                    for nn in range(2):
                        ps = ps512.tile([128, 512], F32, tag="ps512")
                        for hp2 in range(2):
                            nc.tensor.matmul(
                                out=ps[:],
                                lhsT=aoT_sb[:, hp2, st * 128:(st + 1) * 128],
                                rhs=wout_sb[:, hp2, nn * 512:(nn + 1) * 512],
                                start=(hp2 == 0),
                                stop=(hp2 == 1),
                            )
                        ysb = ypool.tile([128, 512], F16, tag="ysb")
                        if (st * 2 + nn) % 2 == 0:
                            nc.scalar.copy(out=ysb[:], in_=ps[:])
                        else:
                            nc.vector.tensor_copy(out=ysb[:], in_=ps[:])
                        nc.sync.dma_start(
                            out=out_d[st * 128:(st + 1) * 128,
                                      nn * 512:(nn + 1) * 512],
                            in_=ysb[:],
                        )

            if stage < 5:
                # debug: dump a qk slice so the NEFF has an output
                nc.sync.dma_start(
                    out=out_d[0:128, :], in_=qk_sb[:, 0, 0:1024]
                )

    nc.compile()
    return nc


_NC = None


def _get_program():
    global _NC
    if _NC is None:
        _NC = _build_program()
    return _NC


def _make_in_maps(x, w_qkv, b_qkv, w_out):
    masks = _build_masks().astype(np.float32)

    in_maps = []
    for c in range(8):
        b, hg = divmod(c, 4)
        cq = 256 * hg
        wqk = np.concatenate(
            [w_qkv[:, cq:cq + 256], w_qkv[:, 1024 + cq:1024 + cq + 256]],
            axis=1,
        ).astype(np.float16)
        bqk = np.empty((128, 4), np.float32)
        bqk[:, 0] = b_qkv[cq:cq + 128] * SCALE
        bqk[:, 1] = b_qkv[cq + 128:cq + 256] * SCALE
        bqk[:, 2] = b_qkv[1024 + cq:1024 + cq + 128]
        bqk[:, 3] = b_qkv[1024 + cq + 128:1024 + cq + 256]
        in_maps.append({
            "xT": np.ascontiguousarray(x[b].T).astype(np.float16),
            "wqk": wqk,
            "wv": w_qkv[:, 2048 + cq:2048 + cq + 256].astype(np.float16),
            "wout": w_out[cq:cq + 256, :].astype(np.float16),
            "bqk": bqk,
            "masks": masks,
        })
    return in_maps


def kernel(x, w_qkv, b_qkv, w_out, b_out):
    x = np.asarray(x, np.float32)
    w_qkv = np.asarray(w_qkv, np.float32)
    b_qkv = np.asarray(b_qkv, np.float32)
    w_out = np.asarray(w_out, np.float32)
    b_out = np.asarray(b_out, np.float32)

    in_maps = _make_in_maps(x, w_qkv, b_qkv, w_out)
    nc = _get_program()
    res = run_bass_kernel_spmd(nc, in_maps, list(range(8)))

    b_v = b_qkv[2048:]
    bias_all = b_out + b_v @ w_out  # folds the (untracked) v-bias
    y = np.empty((B, S, D), np.float32)
    for b in range(B):
        acc = np.zeros((S, D), np.float32)
        for hg in range(4):
            acc += res.results[4 * b + hg]["out"].astype(np.float32)
        y[b] = acc + bias_all
    return y
